# revision 1
# baseline (speedup 1.0000x reference)
"""Trainium2 Bass kernel for nn_AxonalConnections (gnn_message_passing).

Computes out[b,t] = sum_s adjacency[t,s] * mod[b,s],  mod = (1.5*E - 0.5) * spikes,
i.e. a batched mat-vec against a [16384, 16384] adjacency, reshaped to [32,128,128].

Sharding: adjacency row-shard (target dim) across 8 cores; spikes/E replicated;
each core produces out[:, t_shard] — pure output sharding, no collectives.

Two device paths:
  * dense: bf16 GEMM, K=16384 accumulated in fp32 PSUM. Adjacency is host-side
    transposed/cast once so each core streams its [S, T/8] bf16 slab with
    fully-contiguous DMAs (the 1 GiB matrix crossing HBM once is the roofline).
  * sparse: when the adjacency's nonzeros all lie on the 9 conv-pattern
    diagonals (the generator's graph structure), the GEMM is exactly a
    9-tap locally-connected stencil: out[t,b] = sum_k w_k[t]*mod[t+d_k,b].
    Verified exhaustively on the host (nnz match) before use; falls back to
    the dense path for any other adjacency.
"""

import sys

if "/opt/trn_rl_repo" not in sys.path:
    sys.path.insert(0, "/opt/trn_rl_repo")

from contextlib import ExitStack

import ml_dtypes
import numpy as np

B = 32
H = 128
W = 128
S = H * W            # 16384
NCORES = 8
TL = S // NCORES     # 2048 t-columns per core
KC = S // 128        # 128 contraction chunks
P = 128

# sparse path geometry: 3x3 conv neighborhood offsets in flattened index space
DIAG_OFFSETS = [di * W + dj for di in (-1, 0, 1) for dj in (-1, 0, 1)]
PADR = 129           # max |offset|
SLAB = 19 * 128      # padded per-core mod slab rows (2048 + 2*129 -> round up)

_progs = {}


def _build_dense():
    import concourse.tile as tile
    from concourse import bacc, mybir

    nc = bacc.Bacc("TRN2", target_bir_lowering=False, debug=False, num_devices=NCORES)
    f32 = mybir.dt.float32
    bf16 = mybir.dt.bfloat16

    adjt = nc.dram_tensor("adjt", [S, TL], bf16, kind="ExternalInput").ap()
    spt = nc.dram_tensor("spt", [P, KC, B], f32, kind="ExternalInput").ap()
    ef = nc.dram_tensor("ef", [P, KC], f32, kind="ExternalInput").ap()
    outt = nc.dram_tensor("out", [B, TL], f32, kind="ExternalOutput").ap()

    NT = TL // 512  # psum banks used for the output row block

    with tile.TileContext(nc) as tc:
        with ExitStack() as ctx:
            const = ctx.enter_context(tc.tile_pool(name="const", bufs=1))
            adj_pool = ctx.enter_context(tc.tile_pool(name="adj", bufs=10))
            psum = ctx.enter_context(tc.tile_pool(name="psum", bufs=1, space="PSUM"))
            outp = ctx.enter_context(tc.tile_pool(name="outp", bufs=1))

            sp_t = const.tile([P, KC, B], f32)
            nc.sync.dma_start(sp_t[:], spt[:])
            e_t = const.tile([P, KC], f32)
            nc.sync.dma_start(e_t[:], ef[:])
            fac = const.tile([P, KC], f32)
            # fac = 1.5*E - 0.5  (E in {0,1} -> {1.0, -0.5}; exact in bf16 later)
            nc.vector.tensor_scalar(
                fac[:], e_t[:], 1.5, -0.5,
                op0=mybir.AluOpType.mult, op1=mybir.AluOpType.add,
            )
            modt = const.tile([P, KC, B], bf16)
            for k in range(KC):
                nc.vector.tensor_scalar(
                    modt[:, k, :], sp_t[:, k, :], fac[:, k : k + 1], None,
                    op0=mybir.AluOpType.mult,
                )

            pts = [psum.tile([B, 512], f32, name=f"acc{j}") for j in range(NT)]
            for k in range(KC):
                at = adj_pool.tile([P, TL], bf16)
                nc.sync.dma_start(at[:], adjt[k * P : (k + 1) * P, :])
                for j in range(NT):
                    nc.tensor.matmul(
                        pts[j][:],
                        modt[:, k, :],
                        at[:, j * 512 : (j + 1) * 512],
                        start=(k == 0),
                        stop=(k == KC - 1),
                    )

            ot = outp.tile([B, TL], f32)
            for j in range(NT):
                nc.vector.tensor_copy(out=ot[:, j * 512 : (j + 1) * 512], in_=pts[j][:])
            nc.sync.dma_start(outt[:], ot[:])

    nc.compile()
    return nc


def _build_sparse():
    import concourse.tile as tile
    from concourse import bacc, mybir

    nc = bacc.Bacc("TRN2", target_bir_lowering=False, debug=False, num_devices=NCORES)
    f32 = mybir.dt.float32

    # per-core inputs (host pre-sliced, zero-padded at the global boundary):
    #   spsl[r, b]  = spikesT[t0 - PADR + r, b]   r in [0, SLAB)
    #   esl[r]      = E_flat[t0 - PADR + r]
    #   wd[p, j, k] = W9[t0 + j*128 + p, k]       (j t-tile, k tap)
    spsl = nc.dram_tensor("spsl", [SLAB, B], f32, kind="ExternalInput").ap()
    esl = nc.dram_tensor("esl", [SLAB, 1], f32, kind="ExternalInput").ap()
    wd = nc.dram_tensor("wd", [P, TL // P, 9], f32, kind="ExternalInput").ap()
    outt = nc.dram_tensor("out", [TL, B], f32, kind="ExternalOutput").ap()

    mods = nc.dram_tensor("mods", [SLAB, B], f32)  # internal scratch

    NJ = TL // P  # 16 output t-tiles

    with tile.TileContext(nc) as tc:
        with ExitStack() as ctx:
            const = ctx.enter_context(tc.tile_pool(name="const", bufs=1))
            mpool = ctx.enter_context(tc.tile_pool(name="mpool", bufs=6))
            apool = ctx.enter_context(tc.tile_pool(name="apool", bufs=4))

            wdt = const.tile([P, NJ, 9], f32)
            nc.sync.dma_start(wdt[:], wd[:])

            # mod = (1.5E - 0.5) * spikes on the padded slab, bounced via DRAM
            # so the 9 tap reads below can use partition-shifted addressing.
            for c in range(SLAB // P):
                st = mpool.tile([P, B], f32, name="modsrc")
                nc.sync.dma_start(st[:], spsl[c * P : (c + 1) * P, :])
                et = mpool.tile([P, 1], f32, name="modfac")
                nc.sync.dma_start(et[:], esl[c * P : (c + 1) * P, :])
                ft = mpool.tile([P, 1], f32, name="modfac2")
                nc.vector.tensor_scalar(
                    ft[:], et[:], 1.5, -0.5,
                    op0=mybir.AluOpType.mult, op1=mybir.AluOpType.add,
                )
                mt = mpool.tile([P, B], f32, name="modout")
                nc.vector.tensor_scalar(
                    mt[:], st[:], ft[:], None, op0=mybir.AluOpType.mult
                )
                nc.sync.dma_start(mods[c * P : (c + 1) * P, :], mt[:])

            for j in range(NJ):
                acc = None
                for k, d in enumerate(DIAG_OFFSETS):
                    r0 = j * P + PADR + d
                    sh = apool.tile([P, B], f32, name="shift")
                    nc.sync.dma_start(sh[:], mods[r0 : r0 + P, :])
                    nxt = apool.tile([P, B], f32, name=f"acc{k % 2}")
                    if acc is None:
                        nc.vector.tensor_scalar(
                            nxt[:], sh[:], wdt[:, j, k : k + 1], None,
                            op0=mybir.AluOpType.mult,
                        )
                    else:
                        nc.vector.scalar_tensor_tensor(
                            nxt[:], sh[:], wdt[:, j, k : k + 1], acc[:],
                            op0=mybir.AluOpType.mult, op1=mybir.AluOpType.add,
                        )
                    acc = nxt
                nc.sync.dma_start(outt[j * P : (j + 1) * P, :], acc[:])

    nc.compile()
    return nc


def _get_prog(name):
    if name not in _progs:
        _progs[name] = {"dense": _build_dense, "sparse": _build_sparse}[name]()
    return _progs[name]


def _run(nc, in_maps):
    from concourse.bass_utils import run_bass_kernel_spmd

    return run_bass_kernel_spmd(nc, in_maps, core_ids=list(range(NCORES))).results


def _extract_diagonals(adjacency):
    """W9[t, k] = adjacency[t, t + d_k] (0 where out of range).

    Returns (W9, exact) where exact means every nonzero of adjacency lies on
    those 9 diagonals, making the stencil reproduction bit-exact.
    """
    t = np.arange(S)
    W9 = np.zeros((S, 9), np.float32)
    for k, d in enumerate(DIAG_OFFSETS):
        s = t + d
        valid = (s >= 0) & (s < S)
        W9[valid, k] = adjacency[t[valid], s[valid]]
    exact = np.count_nonzero(adjacency) == np.count_nonzero(W9)
    return W9, exact


def _kernel_dense(sp_flat, E_flat, adjacency):
    nc = _get_prog("dense")
    spt = np.ascontiguousarray(sp_flat.T.reshape(KC, P, B).transpose(1, 0, 2))
    ef = np.ascontiguousarray(E_flat.reshape(KC, P).T)
    adj_bf = adjacency.astype(ml_dtypes.bfloat16)
    in_maps = []
    for m in range(NCORES):
        adjt_m = np.ascontiguousarray(adj_bf[m * TL : (m + 1) * TL, :].T)
        in_maps.append({"adjt": adjt_m, "spt": spt, "ef": ef})
    results = _run(nc, in_maps)
    out = np.empty((B, S), np.float32)
    for m in range(NCORES):
        out[:, m * TL : (m + 1) * TL] = results[m]["out"]
    return out


def _kernel_sparse(sp_flat, E_flat, W9):
    nc = _get_prog("sparse")
    spT = sp_flat.T  # [S, B]
    in_maps = []
    for m in range(NCORES):
        t0 = m * TL
        spsl = np.zeros((SLAB, B), np.float32)
        esl = np.zeros((SLAB, 1), np.float32)
        lo, hi = t0 - PADR, t0 - PADR + SLAB
        clo, chi = max(lo, 0), min(hi, S)
        spsl[clo - lo : chi - lo] = spT[clo:chi]
        esl[clo - lo : chi - lo, 0] = E_flat[clo:chi]
        wdm = np.ascontiguousarray(
            W9[t0 : t0 + TL].reshape(TL // P, P, 9).transpose(1, 0, 2)
        )
        in_maps.append({"spsl": spsl, "esl": esl, "wd": wdm})
    results = _run(nc, in_maps)
    out = np.empty((B, S), np.float32)
    for m in range(NCORES):
        out[:, m * TL : (m + 1) * TL] = results[m]["out"].T
    return out


def kernel(spikes, E, adjacency):
    spikes = np.asarray(spikes, np.float32)
    E = np.asarray(E, np.float32)
    adjacency = np.asarray(adjacency, np.float32)
    sp_flat = spikes.reshape(B, S)
    E_flat = E.reshape(S)

    W9, exact = _extract_diagonals(adjacency)
    if exact:
        out = _kernel_sparse(sp_flat, E_flat, W9)
    else:
        out = _kernel_dense(sp_flat, E_flat, adjacency)
    return out.reshape(B, H, W)


# revision 6
# speedup vs baseline: 4.9277x; 4.9277x over previous
"""Trainium2 Bass kernel for nn_AxonalConnections (gnn_message_passing).

Computes out[b,t] = sum_s adjacency[t,s] * mod[b,s],  mod = (1.5*E - 0.5) * spikes,
i.e. a batched mat-vec against a [16384, 16384] adjacency, reshaped to [32,128,128].

Sharding: adjacency row-shard (target dim) across 8 cores; spikes/E replicated;
each core produces out[:, t_shard] — pure output sharding, no collectives.

Two device paths:

* dense: bf16 GEMM, K=16384 accumulated in fp32 PSUM. Adjacency is host-side
  transposed/cast once so each core streams its [S, T/8] bf16 slab with
  fully-contiguous DMAs. The 0.5 GiB bf16 matrix crossing HBM once (~360 GB/s
  per core) and the PE streaming it at 128 lanes/cycle are the two rooflines;
  both sit at ~200 us/core and overlap.

* sparse: when the adjacency's nonzeros all lie on the 9 conv-pattern
  diagonals (the generator's 3x3 message-passing graph), the GEMM is exactly a
  9-tap locally-connected stencil: out[b,t] = sum_k w9[t,k]*mod[b,t+d_k].
  The E-modulation is folded into w9 on the host (exact: the factor is a
  power-of-two scale in {1.0, -0.5}), and each core evaluates the stencil on
  a [4 t-quarters x 32 batch, 512+halo] packed layout where every tap is a
  free-dim AP offset — no per-tap data movement. Structure is verified
  exhaustively on the host (nonzero-count match) before use; any other
  adjacency falls back to the dense path.
"""

import sys

if "/opt/trn_rl_repo" not in sys.path:
    sys.path.insert(0, "/opt/trn_rl_repo")

from contextlib import ExitStack

import ml_dtypes
import numpy as np

B = 32
H = 128
W = 128
S = H * W            # 16384
NCORES = 8
TL = S // NCORES     # 2048 t-columns per core
KC = S // 128        # 128 contraction chunks (dense path)
P = 128

# sparse path geometry: 3x3 conv neighborhood offsets in flattened index space
DIAG_OFFSETS = [di * W + dj for di in (-1, 0, 1) for dj in (-1, 0, 1)]
NTAP = len(DIAG_OFFSETS)
PADR = 129           # max |offset|
NQ = 4               # t-quarters packed on partitions: 4*32 = 128
QT = TL // NQ        # 512 t per quarter
QW = QT + 2 * PADR   # quarter slab width incl. halo

_progs = {}


def _build_dense():
    import concourse.tile as tile
    from concourse import bacc, mybir

    nc = bacc.Bacc("TRN2", target_bir_lowering=False, debug=False, num_devices=NCORES)
    f32 = mybir.dt.float32
    bf16 = mybir.dt.bfloat16

    adjt = nc.dram_tensor("adjt", [S, TL], bf16, kind="ExternalInput").ap()
    spt = nc.dram_tensor("spt", [P, KC, B], f32, kind="ExternalInput").ap()
    ef = nc.dram_tensor("ef", [P, KC], f32, kind="ExternalInput").ap()
    outt = nc.dram_tensor("out", [B, TL], f32, kind="ExternalOutput").ap()

    NT = TL // 512  # psum banks used for the output row block

    with tile.TileContext(nc) as tc:
        with ExitStack() as ctx:
            const = ctx.enter_context(tc.tile_pool(name="const", bufs=1))
            adj_pool = ctx.enter_context(tc.tile_pool(name="adj", bufs=10))
            psum = ctx.enter_context(tc.tile_pool(name="psum", bufs=1, space="PSUM"))
            outp = ctx.enter_context(tc.tile_pool(name="outp", bufs=1))

            sp_t = const.tile([P, KC, B], f32)
            nc.sync.dma_start(sp_t[:], spt[:])
            e_t = const.tile([P, KC], f32)
            nc.sync.dma_start(e_t[:], ef[:])
            fac = const.tile([P, KC], f32)
            # fac = 1.5*E - 0.5  (E in {0,1} -> {1.0, -0.5})
            nc.vector.tensor_scalar(
                fac[:], e_t[:], 1.5, -0.5,
                op0=mybir.AluOpType.mult, op1=mybir.AluOpType.add,
            )
            modt = const.tile([P, KC, B], bf16)
            for k in range(KC):
                nc.vector.tensor_scalar(
                    modt[:, k, :], sp_t[:, k, :], fac[:, k : k + 1], None,
                    op0=mybir.AluOpType.mult,
                )

            pts = [psum.tile([B, 512], f32, name=f"acc{j}") for j in range(NT)]
            for k in range(KC):
                at = adj_pool.tile([P, TL], bf16)
                nc.sync.dma_start(at[:], adjt[k * P : (k + 1) * P, :])
                for j in range(NT):
                    nc.tensor.matmul(
                        pts[j][:],
                        modt[:, k, :],
                        at[:, j * 512 : (j + 1) * 512],
                        start=(k == 0),
                        stop=(k == KC - 1),
                    )

            ot = outp.tile([B, TL], f32)
            for j in range(NT):
                nc.vector.tensor_copy(out=ot[:, j * 512 : (j + 1) * 512], in_=pts[j][:])
            nc.sync.dma_start(outt[:], ot[:])

    nc.compile()
    return nc


def _build_sparse():
    import concourse.tile as tile
    from concourse import bacc, mybir

    nc = bacc.Bacc("TRN2", target_bir_lowering=False, debug=False, num_devices=NCORES)
    f32 = mybir.dt.float32

    # per-core inputs (host pre-packed into the [4 quarters x 32 batch] layout):
    #   spq[32q+b, i] = spikes_flat[b, t0 + q*QT - PADR + i]   (zero-padded at edges)
    #   wq[32q+b, k, i] = wfold[t0 + q*QT + i, k]              (batch-replicated)
    spq = nc.dram_tensor("spq", [P, QW], f32, kind="ExternalInput").ap()
    wq = nc.dram_tensor("wq", [P, NTAP, QT], f32, kind="ExternalInput").ap()
    # packed [32q+b, t] layout; host unpacks to [B, TL]
    outt = nc.dram_tensor("out", [P, QT], f32, kind="ExternalOutput").ap()

    with tile.TileContext(nc) as tc:
        with ExitStack() as ctx:
            pool = ctx.enter_context(tc.tile_pool(name="pool", bufs=1))

            spt = pool.tile([P, QW], f32)
            nc.sync.dma_start(spt[:], spq[:])
            wt = pool.tile([P, NTAP, QT], f32)
            nc.sync.dma_start(wt[:], wq[:])

            acc = None
            for k, d in enumerate(DIAG_OFFSETS):
                sh = spt[:, PADR + d : PADR + d + QT]
                prod = pool.tile([P, QT], f32, name=f"prod{k}")
                nc.vector.tensor_tensor(
                    prod[:], sh, wt[:, k, :], mybir.AluOpType.mult
                )
                if acc is None:
                    acc = prod
                else:
                    nxt = pool.tile([P, QT], f32, name=f"accs{k}")
                    nc.vector.tensor_tensor(
                        nxt[:], acc[:], prod[:], mybir.AluOpType.add
                    )
                    acc = nxt

            nc.sync.dma_start(outt[:], acc[:])

    nc.compile()
    return nc


def _get_prog(name):
    if name not in _progs:
        _progs[name] = {"dense": _build_dense, "sparse": _build_sparse}[name]()
    return _progs[name]


def _run(nc, in_maps, **kwargs):
    from concourse.bass_utils import run_bass_kernel_spmd

    return run_bass_kernel_spmd(nc, in_maps, core_ids=list(range(NCORES)), **kwargs)


def _extract_diagonals(adjacency):
    """W9[t, k] = adjacency[t, t + d_k] (0 where out of range).

    Returns (W9, exact) where exact means every nonzero of adjacency lies on
    those 9 diagonals, making the stencil reproduction of the GEMM exact.
    """
    t = np.arange(S)
    W9 = np.zeros((S, NTAP), np.float32)
    for k, d in enumerate(DIAG_OFFSETS):
        s = t + d
        valid = (s >= 0) & (s < S)
        W9[valid, k] = adjacency[t[valid], s[valid]]
    exact = np.count_nonzero(adjacency) == np.count_nonzero(W9)
    return W9, exact


def _prep_dense_inmaps(sp_flat, E_flat, adjacency):
    spt = np.ascontiguousarray(sp_flat.T.reshape(KC, P, B).transpose(1, 0, 2))
    ef = np.ascontiguousarray(E_flat.reshape(KC, P).T)
    adj_bf = adjacency.astype(ml_dtypes.bfloat16)
    in_maps = []
    for m in range(NCORES):
        adjt_m = np.ascontiguousarray(adj_bf[m * TL : (m + 1) * TL, :].T)
        in_maps.append({"adjt": adjt_m, "spt": spt, "ef": ef})
    return in_maps


def _prep_sparse_inmaps(sp_flat, E_flat, W9):
    # fold the E-modulation into the tap weights: exact because the factor is
    # the power-of-two scale {1.0, -0.5}
    fac = 1.5 * E_flat - 0.5
    t = np.arange(S)
    wfold = np.empty_like(W9)  # [S, 9]
    for k, d in enumerate(DIAG_OFFSETS):
        s = np.clip(t + d, 0, S - 1)
        wfold[:, k] = W9[:, k] * fac[s]

    sp_pad = np.zeros((B, S + 2 * PADR), np.float32)
    sp_pad[:, PADR : PADR + S] = sp_flat

    in_maps = []
    for m in range(NCORES):
        t0 = m * TL
        spq = np.empty((NQ, B, QW), np.float32)
        for q in range(NQ):
            spq[q] = sp_pad[:, t0 + q * QT : t0 + q * QT + QW]
        wslab = wfold[t0 : t0 + TL].reshape(NQ, QT, NTAP).transpose(0, 2, 1)
        wqm = np.broadcast_to(wslab[:, None], (NQ, B, NTAP, QT))
        in_maps.append(
            {
                "spq": spq.reshape(P, QW),
                "wq": np.ascontiguousarray(wqm).reshape(P, NTAP, QT),
            }
        )
    return in_maps


def _gather_out(results):
    out = np.empty((B, S), np.float32)
    for m in range(NCORES):
        r = results[m]["out"]
        if r.shape == (P, QT):  # sparse path: unpack [32q+b, t] -> [b, q*QT+t]
            r = r.reshape(NQ, B, QT).transpose(1, 0, 2).reshape(B, TL)
        out[:, m * TL : (m + 1) * TL] = r
    return out


def kernel(spikes, E, adjacency):
    spikes = np.asarray(spikes, np.float32)
    E = np.asarray(E, np.float32)
    adjacency = np.asarray(adjacency, np.float32)
    sp_flat = spikes.reshape(B, S)
    E_flat = E.reshape(S)

    W9, exact = _extract_diagonals(adjacency)
    if exact:
        in_maps = _prep_sparse_inmaps(sp_flat, E_flat, W9)
        results = _run(_get_prog("sparse"), in_maps).results
    else:
        in_maps = _prep_dense_inmaps(sp_flat, E_flat, adjacency)
        results = _run(_get_prog("dense"), in_maps).results
    return _gather_out(results).reshape(B, H, W)
